# revision 23
# baseline (speedup 1.0000x reference)
"""Trainium2 Bass kernel for nn_Attention_9320079032376.

Full attention block: RMSNorm -> QKV proj -> interleaved RoPE -> GQA causal
attention (32 q heads / 8 kv heads, hd=64) -> out proj.  B=2, S=2048, D=2048.

Sharding: 8 cores = 2 batches x 4 kv-head-pairs.  Core c handles batch c//4
and kv heads {2j, 2j+1} (j = c%4) plus their 8 GQA q-heads.  Host pre-casts
to bf16, pre-transposes x, and pre-permutes weight columns so that:
  - each q "pack" of 2 heads (one per kv head) occupies 128 SBUF partitions,
  - head dims are de-interleaved (evens then odds) so RoPE becomes
    two table multiplies + one partition-swap matmul + one add,
  - the per-token RMSNorm scale r is folded into the RoPE tables (and v).

v2 structure (vs the original baseline):
  - RMSNorm stats come straight from xbT (no separate x load, no PE
    transposes): DVE squares -> ones-matmul column sums -> sqrt+recip.
  - Attention runs transposed (scores^T [kt, qt]) with K=64 row-tiled
    matmul pairs, additive -1e5 causal mask on diagonal 128-blocks, exp on
    ACT with the 1/8 scale fused, denominator rows from a ones-column in v.
  - og is NOT normalized on the attention path.  Each 512-token chunk's
    og (+2 den rows per pack) is AllGathered per chunk so the output
    projection of chunk c overlaps attention of chunk c+1.  1/den is
    broadcast via a partition-replicating SBUF DMA and folded into og with
    DVE muls right before the wo matmuls.
"""
import sys
sys.path.insert(0, "/opt/trn_rl_repo")

import contextlib
import numpy as np
import ml_dtypes

import concourse.bass as bass
import concourse.mybir as mybir
import concourse.tile as tile
from concourse import bacc
from concourse.bass import ts, ds
from concourse.masks import make_identity

BF16 = ml_dtypes.bfloat16
bf16 = mybir.dt.bfloat16
f32 = mybir.dt.float32
AF = mybir.ActivationFunctionType
ALU = mybir.AluOpType

B, S, D = 2, 2048, 2048
HEADS, KV, HD = 32, 8, 64
EPS = 1.1920929e-07
THETA = 10000.0
NCORE = 8
# Mask constant: large enough that exp(0.125*(s+NEG)) ~ 1e-13 (negligible in
# the og/den sums), small enough to stay in the int16-Schraudolph safe range.
NEG = -240.0
# Schraudolph exp in bf16 bit space: exp(0.125*x) = 2^y, y = 0.125*x*log2e;
# i16 = round(y*128 + (127*128 - C)); bitcast(i16) ~ exp with ~2% rms error.
SCH_A = 0.125 * 1.4426950408889634 * 128.0
SCH_B = 127.0 * 128.0 - 7.5

PERM64 = np.concatenate([np.arange(0, 64, 2), np.arange(1, 64, 2)])


# ---------------------------------------------------------------- builder
def build_nc(Sx=S, Dx=D, groups=4, num_devices=8):
    TC = Sx // 512          # q/t chunks of 512
    DT = Dx // 128          # contraction dim tiles
    NT = Sx // 128          # token tiles of 128
    RG = ([[0, 1, 2, 3], [4, 5, 6, 7]] if groups == 4 else [[0]])

    nc = bacc.Bacc("TRN2", target_bir_lowering=False, debug=False,
                   num_devices=num_devices)
    xbT = nc.dram_tensor("xbT", [Dx, Sx], bf16, kind="ExternalInput")
    wq = nc.dram_tensor("wq", [Dx, 512], bf16, kind="ExternalInput")
    wk = nc.dram_tensor("wk", [Dx, 128], bf16, kind="ExternalInput")
    wv = nc.dram_tensor("wv", [Dx, 128], bf16, kind="ExternalInput")
    wo = nc.dram_tensor("wo", [2048, 512], bf16, kind="ExternalInput")
    c128 = nc.dram_tensor("c128", [128, Sx], f32, kind="ExternalInput")
    s128 = nc.dram_tensor("s128", [128, Sx], f32, kind="ExternalInput")
    mneg = nc.dram_tensor("mneg", [128, 128], bf16, kind="ExternalInput")
    perm = nc.dram_tensor("perm", [128, 128], bf16, kind="ExternalInput")
    outT = nc.dram_tensor("outT", [512, Sx], f32, kind="ExternalOutput")
    og_dr = [nc.dram_tensor(f"og_dr{t}", [128, 4, 512], bf16)
             for t in range(TC)]
    og_ag = [nc.dram_tensor(f"og_ag{t}", [128 * groups, 4, 512], bf16)
             for t in range(TC)]

    with tile.TileContext(nc) as tc, contextlib.ExitStack() as ctx:
        const = ctx.enter_context(tc.tile_pool(name="const", bufs=1))
        wpool = ctx.enter_context(tc.tile_pool(name="wpool", bufs=1))
        qkv = ctx.enter_context(tc.tile_pool(name="qkv", bufs=1))
        rwork = ctx.enter_context(tc.tile_pool(name="rwork", bufs=1))

        identb = const.tile([128, 128], bf16)
        make_identity(nc, identb)
        identf = const.tile([128, 128], f32)
        make_identity(nc, identf)
        mnegt = const.tile([128, 128], bf16)
        nc.sync.dma_start(out=mnegt[:], in_=mneg[:])
        permt = const.tile([128, 128], bf16)
        nc.sync.dma_start(out=permt[:], in_=perm[:])
        ones_f = const.tile([1, 128], f32)
        nc.vector.memset(ones_f[:], 1.0)
        ones_b = const.tile([128, 1], bf16)
        nc.vector.memset(ones_b[:], 1.0)
        epsb = const.tile([1, 1], f32)
        nc.vector.memset(epsb[:], float(EPS))

        # persistent sbuf tensors (DMA triggers issued after xbT below —
        # the sync HWDGE ring is FIFO, and xbT gates the stats pipeline)
        wq_sb = wpool.tile([128, DT, 512], bf16)
        wk_sb = wpool.tile([128, DT, 128], bf16)
        wv_sb = wpool.tile([128, DT, 128], bf16)
        wo_sb = wpool.tile([128, 16, 512], bf16)

        crt = rwork.tile([128, Sx], f32, tag="crt")   # cos * r
        srt = rwork.tile([128, Sx], f32, tag="srt")   # sin(+-) * r
        rb_sb = rwork.tile([128, Sx], f32, tag="rb")  # r broadcast to 128 parts

        qT = [qkv.tile([128, Sx], bf16, tag=f"q{i}", name=f"qT{i}") for i in range(4)]
        kT = qkv.tile([128, Sx], bf16, tag="kT")
        # v tiles: [128 tok, 130]: [vA(64) onesA vB(64) onesB]
        v_sb = [qkv.tile([128, 130], bf16, tag=f"v{t}", name=f"vsb{t}") for t in range(NT)]

        # ---------------- stage 0+1: x load, stats, projections -----------
        with tc.tile_pool(name="xbt", bufs=1) as xbt_pool:
            xbT_sb = xbt_pool.tile([128, DT, Sx], bf16)
            xbTr = xbT.rearrange("(dt p) t -> p dt t", p=128)
            for dt in range(DT):
                nc.sync.dma_start(out=xbT_sb[:, dt, :], in_=xbTr[:, dt, :])
            nc.sync.dma_start(out=crt[:], in_=c128[:])
            nc.sync.dma_start(out=srt[:], in_=s128[:])
            nc.sync.dma_start(out=wq_sb[:],
                              in_=wq.rearrange("(dt p) c -> p dt c", p=128))
            nc.sync.dma_start(out=wk_sb[:],
                              in_=wk.rearrange("(dt p) c -> p dt c", p=128))
            nc.sync.dma_start(out=wv_sb[:],
                              in_=wv.rearrange("(dt p) c -> p dt c", p=128))
            nc.sync.dma_start(out=wo_sb[:],
                              in_=wo.rearrange("(s p) c -> p s c", p=128))

            # stats: squares (DVE, bf16) + ones-matmul column sums per chunk
            with tc.tile_pool(name="sq", bufs=3) as sq_pool, \
                 tc.tile_pool(name="st_ps", bufs=4, space="PSUM") as st_ps, \
                 tc.tile_pool(name="rb_ps", bufs=2, space="PSUM") as rb_psp, \
                 tc.tile_pool(name="rrow", bufs=2) as rrow_pool:
                ssqs = [st_ps.tile([1, 512], f32, tag=f"ssq{t}",
                                   name=f"ssq{t}", bufs=1) for t in range(TC)]
                for dt in range(DT):
                    sqt = sq_pool.tile([128, Sx], bf16, tag="sq")
                    nc.vector.tensor_mul(sqt[:], xbT_sb[:, dt, :],
                                         xbT_sb[:, dt, :])
                    for tcc in range(TC):
                        nc.tensor.matmul(ssqs[tcc][:], ones_b[:],
                                         sqt[:, ts(tcc, 512)],
                                         start=(dt == 0), stop=(dt == DT - 1))
                for tcc in range(TC):
                    sl = ts(tcc, 512)
                    sqv = rrow_pool.tile([1, 512], f32, tag="sqv")
                    nc.scalar.activation(sqv[:], ssqs[tcc][:], AF.Sqrt,
                                         bias=epsb[:], scale=float(1.0 / Dx))
                    r_row = rrow_pool.tile([1, 512], f32, tag="rrow")
                    nc.vector.reciprocal(r_row[:], sqv[:])
                    # broadcast r to 128 partitions; fold into tables
                    rbp = rb_psp.tile([128, 512], f32, tag="rb")
                    nc.tensor.matmul(rbp[:], ones_f[:], r_row[:],
                                     start=True, stop=True)
                    nc.vector.tensor_copy(rb_sb[:, sl], rbp[:])
                    nc.vector.tensor_mul(crt[:, sl], crt[:, sl], rbp[:])
                    nc.vector.tensor_mul(srt[:, sl], srt[:, sl], rbp[:])

            # ---------------- projections + rope + v ----------------------
            with tc.tile_pool(name="pj_ps", bufs=4, space="PSUM") as pj_ps, \
                 tc.tile_pool(name="sw_ps", bufs=2, space="PSUM") as sw_ps, \
                 tc.tile_pool(name="vt_ps", bufs=2, space="PSUM") as vt_ps, \
                 tc.tile_pool(name="tmp", bufs=6) as tmp:
                for tcc in range(TC):
                    sl = ts(tcc, 512)
                    for pk in range(6):          # 0-3 q packs, 4 k, 5 v
                        pj = pj_ps.tile([128, 512], f32, tag="pj")
                        for dt in range(DT):
                            if pk < 4:
                                lhs = wq_sb[:, dt, ts(pk, 128)]
                            elif pk == 4:
                                lhs = wk_sb[:, dt, :]
                            else:
                                lhs = wv_sb[:, dt, :]
                            nc.tensor.matmul(pj[:], lhs, xbT_sb[:, dt, sl],
                                             start=(dt == 0), stop=(dt == DT - 1))
                        if pk < 5:
                            tmpc = tmp.tile([128, 512], bf16, tag="tmpc")
                            tmps = tmp.tile([128, 512], bf16, tag="tmps")
                            nc.vector.tensor_mul(tmpc[:], pj[:], crt[:, sl])
                            nc.vector.tensor_mul(tmps[:], pj[:], srt[:, sl])
                            swp = sw_ps.tile([128, 512], f32, tag="sw")
                            nc.tensor.matmul(swp[:], permt[:], tmps[:],
                                             start=True, stop=True)
                            dest = qT[pk] if pk < 4 else kT
                            nc.vector.tensor_add(dest[:, sl], tmpc[:], swp[:])
                        else:
                            vsc = tmp.tile([128, 512], f32, tag="vsc")
                            nc.vector.tensor_mul(vsc[:], pj[:], rb_sb[:, sl])
                            for st in range(4):
                                tt = 4 * tcc + st
                                vp = vt_ps.tile([128, 128], f32, tag="vt")
                                nc.tensor.transpose(vp[:], vsc[:, ts(st, 128)],
                                                    identf[:])
                                nc.vector.tensor_copy(v_sb[tt][:, 0:64], vp[:, 0:64])
                                nc.vector.tensor_copy(v_sb[tt][:, 65:129], vp[:, 64:128])
                                nc.vector.memset(v_sb[tt][:, 64:65], 1.0)
                                nc.vector.memset(v_sb[tt][:, 129:130], 1.0)

        # ---------------- stage 2+3: attention, chunked AG, out proj ------
        with tc.tile_pool(name="s_ps", bufs=2, space="PSUM") as s_ps_pool, \
             tc.tile_pool(name="og_ps", bufs=3, space="PSUM") as og_ps_pool, \
             tc.tile_pool(name="o_ps", bufs=1, space="PSUM") as o_ps_pool, \
             tc.tile_pool(name="att", bufs=3) as att, \
             tc.tile_pool(name="ogo", bufs=4) as ogo, \
             tc.tile_pool(name="ogsb", bufs=2) as ogsb_pool, \
             tc.tile_pool(name="osb", bufs=2) as osb:
            og_sbs = []

            def emit_outproj(ti, og_sb, GT):
                for oc in range(4):
                    ops = o_ps_pool.tile([128, 512], f32, tag="o")
                    for s_ in range(GT):
                        nc.tensor.matmul(ops[:], wo_sb[:, s_, ts(oc, 128)],
                                         og_sb[:, s_, :],
                                         start=(s_ == 0), stop=(s_ == GT - 1))
                    ot_sb = osb.tile([128, 512], f32, tag="otsb")
                    nc.vector.tensor_copy(ot_sb[:], ops[:])
                    nc.gpsimd.dma_start(out=outT[ts(oc, 128), ts(ti, 512)],
                                        in_=ot_sb[:])

            for tcc in range(TC):
                qsl = ts(tcc, 512)
                nkt = (tcc + 1) * 4
                for pk in range(4):
                    og_a = og_ps_pool.tile([128, 512], f32, tag="og")
                    og_b = og_ps_pool.tile([128, 512], f32, tag="og")
                    for kt in range(nkt):
                        kr = kt - 4 * tcc     # >=0 on diagonal tiles
                        sp = s_ps_pool.tile([128, 1024], f32, tag="s")
                        pT = att.tile([128, 1024], bf16, tag="pT")
                        dve_b = (kt + pk) % 2 == 0   # head B exp on DVE

                        def expA(dst, src):
                            nc.scalar.activation(dst, src, AF.Exp, scale=0.125)

                        def expB(dst, src):
                            if dve_b:
                                nc.vector.tensor_scalar(
                                    out=dst.bitcast(mybir.dt.int16), in0=src,
                                    scalar1=SCH_A, scalar2=SCH_B,
                                    op0=ALU.mult, op1=ALU.add)
                            else:
                                nc.scalar.activation(dst, src, AF.Exp,
                                                     scale=0.125)

                        if kr < 0:
                            nc.tensor.matmul(sp[:, 0:512],
                                             kT[0:64, ts(kt, 128)],
                                             qT[pk][0:64, qsl],
                                             start=True, stop=True,
                                             tile_position=(0, 0))
                            nc.tensor.matmul(sp[:, 512:1024],
                                             kT[64:128, ts(kt, 128)],
                                             qT[pk][64:128, qsl],
                                             start=True, stop=True,
                                             tile_position=(64, 0))
                            expA(pT[:, 0:512], sp[:, 0:512])
                            expB(pT[:, 512:1024], sp[:, 512:1024])
                            nc.tensor.matmul(og_a[0:65, :], v_sb[kt][:, 0:65],
                                             pT[:, 0:512],
                                             start=(kt == 0), stop=(kt == nkt - 1))
                            nc.tensor.matmul(og_b[0:65, :], v_sb[kt][:, 65:130],
                                             pT[:, 512:1024],
                                             start=(kt == 0), stop=(kt == nkt - 1))
                        else:
                            c0 = 128 * kr     # first valid q col in chunk
                            w = 512 - c0
                            nc.tensor.matmul(sp[:, ds(c0, w)],
                                             kT[0:64, ts(kt, 128)],
                                             qT[pk][0:64, ds(512 * tcc + c0, w)],
                                             start=True, stop=True,
                                             tile_position=(0, 0))
                            nc.tensor.matmul(sp[:, ds(512 + c0, w)],
                                             kT[64:128, ts(kt, 128)],
                                             qT[pk][64:128, ds(512 * tcc + c0, w)],
                                             start=True, stop=True,
                                             tile_position=(64, 0))
                            nc.vector.tensor_add(sp[:, ds(c0, 128)],
                                                 sp[:, ds(c0, 128)], mnegt[:])
                            nc.vector.tensor_add(sp[:, ds(512 + c0, 128)],
                                                 sp[:, ds(512 + c0, 128)],
                                                 mnegt[:])
                            expA(pT[:, ds(c0, w)], sp[:, ds(c0, w)])
                            expB(pT[:, ds(512 + c0, w)], sp[:, ds(512 + c0, w)])
                            nc.tensor.matmul(og_a[0:65, ds(c0, w)],
                                             v_sb[kt][:, 0:65], pT[:, ds(c0, w)],
                                             start=(kt == 0), stop=(kt == nkt - 1))
                            nc.tensor.matmul(og_b[0:65, ds(c0, w)],
                                             v_sb[kt][:, 65:130],
                                             pT[:, ds(512 + c0, w)],
                                             start=(kt == 0), stop=(kt == nkt - 1))
                    # normalize: 1/den broadcast via ones-matmul, then muls
                    rden_a = ogo.tile([1, 512], f32, tag="rdena")
                    rden_b = ogo.tile([1, 512], f32, tag="rdenb")
                    nc.vector.reciprocal(rden_a[:], og_a[64:65, :])
                    nc.vector.reciprocal(rden_b[:], og_b[64:65, :])
                    dps = o_ps_pool.tile([128, 512], f32, tag="o")
                    nc.tensor.matmul(dps[0:64, :], ones_f[0:1, 0:64],
                                     rden_a[:], start=True, stop=True)
                    nc.tensor.matmul(dps[64:128, :], ones_f[0:1, 0:64],
                                     rden_b[:], start=True, stop=True)
                    bc = ogo.tile([128, 512], f32, tag="bc")
                    nc.vector.tensor_copy(bc[:], dps[:])
                    og_out = ogo.tile([128, 512], bf16, tag="ogout")
                    nc.vector.tensor_mul(og_out[0:64, :], og_a[0:64, :],
                                         bc[0:64, :])
                    nc.vector.tensor_mul(og_out[64:128, :], og_b[0:64, :],
                                         bc[64:128, :])
                    nc.sync.dma_start(out=og_dr[tcc][:, pk, :], in_=og_out[:])

                # ---- chunked AllGather of og; og_sb reload on the SWDGE
                # ring (gpsimd) so its wait never blocks the sync FIFO ----
                if groups > 1:
                    nc.gpsimd.collective_compute(
                        "AllGather", ALU.bypass, replica_groups=RG,
                        ins=[og_dr[tcc][:]], outs=[og_ag[tcc][:]])
                    og_src = og_ag[tcc]
                    NR = 4
                else:
                    og_src = og_dr[tcc]
                    NR = 1
                GT = 4 * NR
                og_sb = ogsb_pool.tile([128, GT, 512], bf16, tag="ogsb",
                                       name=f"ogsb{tcc}")
                nc.gpsimd.dma_start(
                    out=og_sb[:].rearrange("p (r k) t -> p r k t", r=NR),
                    in_=og_src.rearrange("(r p) k t -> p r k t", p=128))
                og_sbs.append(og_sb)

                # out-proj for chunk tcc-1 (traced AFTER chunk tcc's
                # attention so its PE-queue entries don't head-of-line
                # block the next chunk while the AllGather completes)
                if tcc >= 1:
                    emit_outproj(tcc - 1, og_sbs[tcc - 1], GT)
            emit_outproj(TC - 1, og_sbs[TC - 1], GT)
    nc.compile()
    return nc


# ---------------------------------------------------------------- host prep
def _rope_tables(Sx):
    f = np.arange(32)
    invf = THETA ** (-2.0 * f / 64.0)
    t = np.arange(Sx, dtype=np.float64)
    ang = t[None, :] * invf[:, None]
    c = np.tile(np.cos(ang), (4, 1)).astype(np.float32)
    sgn = np.concatenate([np.ones(32), -np.ones(32)] * 2)[:, None]
    s = (np.tile(np.sin(ang), (4, 1)) * sgn).astype(np.float32)
    return c, s


def _mask_neg():
    p = np.arange(128)
    return np.where(p[None, :] >= p[:, None], 0.0, NEG).astype(BF16)


def _perm128():
    m = np.arange(128)
    sw = np.where((m % 64) < 32, m + 32, m - 32)
    P = np.zeros((128, 128), np.float32)
    P[sw, m] = 1.0     # P[k, m] = 1 iff k == swap(m)
    return P.astype(BF16)


def prep_core_inputs(x, w_norm, wq, wk, wv, wo, c):
    j = c % 4
    b = c // 4
    wn = w_norm.astype(np.float32)[:, None]
    xbT = np.ascontiguousarray(x[b].astype(BF16).T)
    cols_q = []
    for i in range(4):
        hA, hB = 2 * j + 8 * i, 2 * j + 1 + 8 * i
        cols_q += list(64 * hA + PERM64) + list(64 * hB + PERM64)
    wq_c = np.ascontiguousarray((wn * wq)[:, cols_q]).astype(BF16)
    cols_k = list(64 * (2 * j) + PERM64) + list(64 * (2 * j + 1) + PERM64)
    wk_c = np.ascontiguousarray((wn * wk)[:, cols_k]).astype(BF16)
    cols_v = list(64 * (2 * j) + np.arange(64)) + list(64 * (2 * j + 1) + np.arange(64))
    wv_c = np.ascontiguousarray((wn * wv)[:, cols_v]).astype(BF16)
    # wo rows: contraction index s*128+p -> og row of slot s=(r,pk), part p:
    # head h = 2*(s//4) + 8*(s%4) + p//64, dim d = p%64 -> wo row 64h+d
    idx = np.arange(2048)
    s_, p_ = idx // 128, idx % 128
    hh = 2 * (s_ // 4) + 8 * (s_ % 4) + p_ // 64
    rows = 64 * hh + (p_ % 64)
    wo_c = np.ascontiguousarray(wo[rows][:, 512 * j:512 * (j + 1)]).astype(BF16)
    c128, s128 = _rope_tables(x.shape[1])
    return {"xbT": xbT, "wq": wq_c, "wk": wk_c, "wv": wv_c,
            "wo": wo_c, "c128": c128, "s128": s128, "mneg": _mask_neg(),
            "perm": _perm128()}


_NC_CACHE = {}


def kernel(x, w_norm, wq, wk, wv, wo):
    x = np.asarray(x); w_norm = np.asarray(w_norm)
    wq = np.asarray(wq); wk = np.asarray(wk)
    wv = np.asarray(wv); wo = np.asarray(wo)
    if "nc" not in _NC_CACHE:
        _NC_CACHE["nc"] = build_nc(S, D, groups=4, num_devices=8)
    nc = _NC_CACHE["nc"]
    in_maps = [prep_core_inputs(x, w_norm, wq, wk, wv, wo, c) for c in range(NCORE)]
    from concourse.bass_utils import run_bass_kernel_spmd
    res = run_bass_kernel_spmd(nc, in_maps, core_ids=list(range(NCORE)))
    out = np.zeros((B, S, D), np.float32)
    for c in range(NCORE):
        b, j = c // 4, c % 4
        out[b, :, 512 * j:512 * (j + 1)] = res.results[c]["outT"].T
    return out
